# revision 12
# baseline (speedup 1.0000x reference)
"""Clopath STDP step on 8 Trainium2 NeuronCores.

Math (per batch b):
  U_pot[b,e,o] = sum_d xbar_pre[d,b,e] * dmap[d,e,o]
  U_dep[b,e,o] = sum_d Xd[d,b,e]       * dmap[d,e,o]
  gp[b,o] = Xpost[b,o] * relu(u_pot[b,o]);  gd[b,o] = relu(u_dep[b,o])
  W_new = clamp(W + A_p*gp*U_pot - A_d*gd*U_dep, 0, wmax)
  plus three exponential-filter updates (xbar_pre, u_pot, u_dep).

Sharding: batch axis across 8 cores (4 batches/core); dmap/A_p/A_d replicated.

Device mapping per core, for each (b, e-chunk of 128):
  - The d-contraction runs on the TensorEngine as 8 accumulating matmuls with a
    *diagonal* stationary operand diag(xbar[d,b,echunk]) against moving dmap[d]
    tiles ([128e x 512o] per PSUM bank), in float32r (1 row/cycle, ~2e-4 rel).
  - Gates are broadcast along partitions via DMA; A*gate products on GPSIMD;
    PSUM-consuming multiply/add/clamp chain on DVE.
"""
import numpy as np
from contextlib import ExitStack

D, B, PRE, POST = 8, 32, 1024, 1024
NCORES = 8
BLOC = B // NCORES  # 4 batches per core
ECH = PRE // 128    # 8 e-chunks
OH = 2              # two 512-wide o-halves per PSUM bank

ALPHA_P = float(np.exp(-1.0 / 10.0))
ALPHA_D = float(np.exp(-1.0 / 20.0))
ALPHA_X = float(np.exp(-1.0 / 15.0))

_CACHE = {}


def _build_program(use_wmax: bool, mm_dtype_name: str = "float32r"):
    import concourse.bass as bass
    import concourse.bacc as bacc
    import concourse.mybir as mybir
    import concourse.tile as tile

    F32 = mybir.dt.float32
    I32 = mybir.dt.int32
    MMDT = getattr(mybir.dt, mm_dtype_name)
    Alu = mybir.AluOpType
    Act = mybir.ActivationFunctionType

    nc = bacc.Bacc("TRN2", target_bir_lowering=False, debug=False,
                   num_devices=NCORES)

    # ---- I/O ----
    xd_in = nc.declare_dram_parameter("Xd", [D, BLOC, PRE], F32, isOutput=False)
    xpost_in = nc.declare_dram_parameter("Xpost", [BLOC, POST], F32, isOutput=False)
    vpost_in = nc.declare_dram_parameter("Vpost", [BLOC, POST], F32, isOutput=False)
    w_in = nc.declare_dram_parameter("W", [BLOC, PRE, POST], F32, isOutput=False)
    xbar_in = nc.declare_dram_parameter("xbar_pre", [D, BLOC, PRE], F32, isOutput=False)
    upot_in = nc.declare_dram_parameter("u_pot", [BLOC, POST], F32, isOutput=False)
    udep_in = nc.declare_dram_parameter("u_dep", [BLOC, POST], F32, isOutput=False)
    dmap_in = nc.declare_dram_parameter("dmap", [D, PRE, POST], MMDT, isOutput=False)
    ap_in = nc.declare_dram_parameter("A_p", [PRE, POST], F32, isOutput=False)
    ad_in = nc.declare_dram_parameter("A_d", [PRE, POST], F32, isOutput=False)
    if use_wmax:
        wmax_in = nc.declare_dram_parameter("wmax", [PRE, POST], F32, isOutput=False)

    wn_out = nc.declare_dram_parameter("W_new", [BLOC, PRE, POST], F32, isOutput=True)
    xbarn_out = nc.declare_dram_parameter("xbar_new", [D, BLOC, PRE], F32, isOutput=True)
    upn_out = nc.declare_dram_parameter("u_pot_new", [BLOC, POST], F32, isOutput=True)
    udn_out = nc.declare_dram_parameter("u_dep_new", [BLOC, POST], F32, isOutput=True)

    # bounce buffer for partition-broadcasting the gate rows
    gates_dram = nc.dram_tensor("gates_bounce", [2 * BLOC, POST], F32)

    with ExitStack() as ctx:
        tc = ctx.enter_context(tile.TileContext(nc))
        const = ctx.enter_context(tc.tile_pool(name="const", bufs=1))
        small = ctx.enter_context(tc.tile_pool(name="small", bufs=1))
        dmapp = ctx.enter_context(tc.tile_pool(name="dmapp", bufs=2))
        apool = ctx.enter_context(tc.tile_pool(name="apool", bufs=2))
        wpool = ctx.enter_context(tc.tile_pool(name="wpool", bufs=2))
        diagp = ctx.enter_context(tc.tile_pool(name="diagp", bufs=68))
        pdp = ctx.enter_context(tc.tile_pool(name="pdp", bufs=2))
        tp = ctx.enter_context(tc.tile_pool(name="tp", bufs=2))
        wnp = ctx.enter_context(tc.tile_pool(name="wnp", bufs=2))
        psum = ctx.enter_context(tc.tile_pool(name="psum", bufs=3, space="PSUM"))

        # ---- phase A: small tensors, filters, gates, transposes ----
        xpost_t = small.tile([BLOC, POST], F32, tag="xpost")
        vpost_t = small.tile([BLOC, POST], F32, tag="vpost")
        upot_t = small.tile([BLOC, POST], F32, tag="upot")
        udep_t = small.tile([BLOC, POST], F32, tag="udep")
        xbar_f = const.tile([D * BLOC, PRE], F32, tag="xbar_f")
        xd_f = const.tile([D * BLOC, PRE], F32, tag="xd_f")
        nc.sync.dma_start(out=xpost_t, in_=xpost_in[:, :])
        nc.sync.dma_start(out=vpost_t, in_=vpost_in[:, :])
        nc.sync.dma_start(out=upot_t, in_=upot_in[:, :])
        nc.sync.dma_start(out=udep_t, in_=udep_in[:, :])
        nc.sync.dma_start(out=xbar_f, in_=xbar_in[:, :, :].rearrange("d b e -> (d b) e"))
        nc.sync.dma_start(out=xd_f, in_=xd_in[:, :, :].rearrange("d b e -> (d b) e"))

        # exponential filters: new = alpha*old + (1-alpha)*x
        tmp = small.tile([BLOC, POST], F32, tag="ftmp")
        upn_t = small.tile([BLOC, POST], F32, tag="upn")
        udn_t = small.tile([BLOC, POST], F32, tag="upn")
        nc.vector.tensor_scalar(out=tmp, in0=vpost_t, scalar1=1.0 - ALPHA_P,
                                scalar2=None, op0=Alu.mult)
        nc.vector.scalar_tensor_tensor(out=upn_t, in0=upot_t, scalar=ALPHA_P,
                                       in1=tmp, op0=Alu.mult, op1=Alu.add)
        nc.sync.dma_start(out=upn_out[:, :], in_=upn_t)
        tmp2 = small.tile([BLOC, POST], F32, tag="ftmp")
        nc.vector.tensor_scalar(out=tmp2, in0=vpost_t, scalar1=1.0 - ALPHA_D,
                                scalar2=None, op0=Alu.mult)
        nc.vector.scalar_tensor_tensor(out=udn_t, in0=udep_t, scalar=ALPHA_D,
                                       in1=tmp2, op0=Alu.mult, op1=Alu.add)
        nc.sync.dma_start(out=udn_out[:, :], in_=udn_t)

        xtmp = const.tile([D * BLOC, PRE], F32, tag="xtmp")
        nc.vector.tensor_scalar(out=xtmp, in0=xd_f, scalar1=1.0 - ALPHA_X,
                                scalar2=None, op0=Alu.mult)
        xbn_t = const.tile([D * BLOC, PRE], F32, tag="xbn")
        nc.vector.scalar_tensor_tensor(out=xbn_t, in0=xbar_f, scalar=ALPHA_X,
                                       in1=xtmp, op0=Alu.mult, op1=Alu.add)
        nc.sync.dma_start(out=xbarn_out[:, :, :].rearrange("d b e -> (d b) e"), in_=xbn_t)

        # gates gp = Xpost*relu(u_pot), gd = relu(u_dep)
        relu_p = small.tile([BLOC, POST], F32, tag="ftmp")
        gp_t = small.tile([BLOC, POST], F32, tag="gp")
        gd_t = small.tile([BLOC, POST], F32, tag="gd")
        nc.vector.tensor_scalar(out=relu_p, in0=upot_t, scalar1=0.0, scalar2=None,
                                op0=Alu.max)
        nc.vector.tensor_tensor(out=gp_t, in0=xpost_t, in1=relu_p, op=Alu.mult)
        nc.vector.tensor_scalar(out=gd_t, in0=udep_t, scalar1=0.0, scalar2=None,
                                op0=Alu.max)
        nc.sync.dma_start(out=gates_dram[0:BLOC, :], in_=gp_t)
        nc.sync.dma_start(out=gates_dram[BLOC:2 * BLOC, :], in_=gd_t)

        # broadcast each gate row to all 128 partitions (resident all kernel)
        gpB = []
        gdB = []
        for b in range(BLOC):
            gpb = const.tile([128, POST], F32, tag=f"gpB{b}")
            gdb = const.tile([128, POST], F32, tag=f"gdB{b}")
            src_p = gates_dram[b:b + 1, :]
            src_d = gates_dram[BLOC + b:BLOC + b + 1, :]
            nc.sync.dma_start(out=gpb, in_=bass.AP(
                tensor=src_p.tensor, offset=src_p.offset, ap=[[0, 128], [1, POST]]))
            nc.sync.dma_start(out=gdb, in_=bass.AP(
                tensor=src_d.tensor, offset=src_d.offset, ap=[[0, 128], [1, POST]]))
            gpB.append(gpb)
            gdB.append(gdb)

        # diagonal mask and 32x32 identity
        iota_t = const.tile([128, 128], I32, tag="iota")
        mask_t = const.tile([128, 128], F32, tag="mask")
        nc.gpsimd.iota(iota_t, pattern=[[1, 128]], base=0, channel_multiplier=-1)
        nc.vector.tensor_scalar(out=mask_t, in0=iota_t, scalar1=0, scalar2=None,
                                op0=Alu.is_equal)
        iota32 = const.tile([32, 32], I32, tag="iota32")
        ident = const.tile([32, 32], F32, tag="ident")
        nc.gpsimd.iota(iota32, pattern=[[1, 32]], base=0, channel_multiplier=-1)
        nc.vector.tensor_scalar(out=ident, in0=iota32, scalar1=0, scalar2=None,
                                op0=Alu.is_equal)

        # transpose xbar/xd to [e, (d,b)] layout, per e-chunk
        xbarT = []
        xdT = []
        with tc.tile_pool(name="pst", bufs=2, space="PSUM") as pst:
            for ech in range(ECH):
                pt = pst.tile([128, D * BLOC], F32, tag="ptrans")
                nc.tensor.transpose(pt, xbar_f[:, ech * 128:(ech + 1) * 128], ident)
                xt = const.tile([128, D * BLOC], F32, tag=f"xbarT{ech}")
                nc.vector.tensor_copy(out=xt, in_=pt)
                xbarT.append(xt)
                pt2 = pst.tile([128, D * BLOC], F32, tag="ptrans")
                nc.tensor.transpose(pt2, xd_f[:, ech * 128:(ech + 1) * 128], ident)
                xt2 = const.tile([128, D * BLOC], F32, tag=f"xdT{ech}")
                nc.vector.tensor_copy(out=xt2, in_=pt2)
                xdT.append(xt2)

        # ---- phase B: main loop ----
        for ech in range(ECH):
            e0 = ech * 128
            # diag tiles are built lazily inside the oh=0 b-loop (so builds
            # overlap the previous batch's matmuls) and persist through oh=1
            diags_pot = [[None] * D for _ in range(BLOC)]
            diags_dep = [[None] * D for _ in range(BLOC)]

            for oh in range(OH):
                o0 = oh * 512
                dmap_t = dmapp.tile([128, D, 512], MMDT, tag="dmap")
                nc.sync.dma_start(
                    out=dmap_t,
                    in_=dmap_in[:, e0:e0 + 128, o0:o0 + 512].transpose([1, 0, 2]))
                a_p = apool.tile([128, 512], F32, tag="a_p")
                a_d = apool.tile([128, 512], F32, tag="a_d")
                nc.sync.dma_start(out=a_p, in_=ap_in[e0:e0 + 128, o0:o0 + 512])
                nc.sync.dma_start(out=a_d, in_=ad_in[e0:e0 + 128, o0:o0 + 512])
                if use_wmax:
                    wm_t = apool.tile([128, 512], F32, tag="wm")
                    nc.sync.dma_start(out=wm_t, in_=wmax_in[e0:e0 + 128, o0:o0 + 512])

                # one batched DMA for W in and W_new out per (ech, oh)
                w_all = wpool.tile([128, BLOC, 512], F32, tag="w")
                nc.sync.dma_start(
                    out=w_all,
                    in_=w_in[:, e0:e0 + 128, o0:o0 + 512].transpose([1, 0, 2]))
                wn_all = wnp.tile([128, BLOC, 512], F32, tag="wn")

                for b in range(BLOC):
                    if oh == 0:
                        for d in range(D):
                            col = d * BLOC + b
                            dg_p = diagp.tile([128, 128], MMDT, tag="diag")
                            dg_d = diagp.tile([128, 128], MMDT, tag="diag")
                            nc.scalar.activation(
                                out=dg_p, in_=mask_t, func=Act.Copy,
                                scale=xbarT[ech][:, col:col + 1])
                            nc.scalar.activation(
                                out=dg_d, in_=mask_t, func=Act.Copy,
                                scale=xdT[ech][:, col:col + 1])
                            diags_pot[b][d] = dg_p
                            diags_dep[b][d] = dg_d

                    u_pot = psum.tile([128, 512], F32, tag="u_pot")
                    u_dep = psum.tile([128, 512], F32, tag="u_dep")
                    for d in range(D):
                        nc.tensor.matmul(u_pot, diags_pot[b][d], dmap_t[:, d, :],
                                         start=(d == 0), stop=(d == D - 1))
                    for d in range(D):
                        nc.tensor.matmul(u_dep, diags_dep[b][d], dmap_t[:, d, :],
                                         start=(d == 0), stop=(d == D - 1))

                    p_t = pdp.tile([128, 512], F32, tag="p")
                    d_t = pdp.tile([128, 512], F32, tag="d")
                    nc.gpsimd.tensor_tensor(out=p_t, in0=a_p,
                                            in1=gpB[b][:, o0:o0 + 512], op=Alu.mult)
                    nc.gpsimd.tensor_tensor(out=d_t, in0=a_d,
                                            in1=gdB[b][:, o0:o0 + 512], op=Alu.mult)

                    t_pot = tp.tile([128, 512], F32, tag="t_pot")
                    t_dep = tp.tile([128, 512], F32, tag="t_dep")
                    nc.vector.tensor_tensor(out=t_pot, in0=u_pot, in1=p_t, op=Alu.mult)
                    nc.vector.tensor_tensor(out=t_dep, in0=u_dep, in1=d_t, op=Alu.mult)
                    nc.vector.tensor_tensor(out=t_pot, in0=t_pot, in1=w_all[:, b, :],
                                            op=Alu.add)
                    nc.vector.tensor_tensor(out=t_pot, in0=t_pot, in1=t_dep,
                                            op=Alu.subtract)
                    if use_wmax:
                        nc.vector.tensor_scalar(out=t_pot, in0=t_pot, scalar1=0.0,
                                                scalar2=None, op0=Alu.max)
                        nc.vector.tensor_tensor(out=wn_all[:, b, :], in0=t_pot,
                                                in1=wm_t, op=Alu.min)
                    else:
                        nc.vector.tensor_scalar(out=wn_all[:, b, :], in0=t_pot,
                                                scalar1=0.0, scalar2=1.0,
                                                op0=Alu.max, op1=Alu.min)

                nc.sync.dma_start(
                    out=wn_out[:, e0:e0 + 128, o0:o0 + 512].transpose([1, 0, 2]),
                    in_=wn_all)

    nc.compile()
    return nc


def _get_program(use_wmax: bool):
    key = ("prog", use_wmax)
    if key not in _CACHE:
        _CACHE[key] = _build_program(use_wmax)
    return _CACHE[key]


def _run(inputs, trace=False):
    from concourse.bass_utils import run_bass_kernel_spmd

    f32 = lambda a: np.ascontiguousarray(np.asarray(a), dtype=np.float32)
    Xd = f32(inputs["Xd"]); Xpost = f32(inputs["Xpost"]); Vpost = f32(inputs["Vpost"])
    W = f32(inputs["W"]); xbar = f32(inputs["xbar_pre"])
    u_pot = f32(inputs["u_pot"]); u_dep = f32(inputs["u_dep"])
    dmap = f32(inputs["dmap"]); A_p = f32(inputs["A_p"]); A_d = f32(inputs["A_d"])
    wmax = f32(inputs["wmax"])

    use_wmax = not bool(np.all(wmax == 1.0))
    nc = _get_program(use_wmax)

    in_maps = []
    for c in range(NCORES):
        bs = slice(c * BLOC, (c + 1) * BLOC)
        m = {
            "Xd": np.ascontiguousarray(Xd[:, bs, :]),
            "Xpost": np.ascontiguousarray(Xpost[bs]),
            "Vpost": np.ascontiguousarray(Vpost[bs]),
            "W": np.ascontiguousarray(W[bs]),
            "xbar_pre": np.ascontiguousarray(xbar[:, bs, :]),
            "u_pot": np.ascontiguousarray(u_pot[bs]),
            "u_dep": np.ascontiguousarray(u_dep[bs]),
            "dmap": dmap,
            "A_p": A_p,
            "A_d": A_d,
        }
        if use_wmax:
            m["wmax"] = wmax
        in_maps.append(m)

    out = run_bass_kernel_spmd(nc, in_maps, core_ids=list(range(NCORES)),
                               trace=trace)
    res = out.results
    W_new = np.concatenate([r["W_new"] for r in res], axis=0)
    xbar_new = np.concatenate([r["xbar_new"] for r in res], axis=1)
    u_pot_new = np.concatenate([r["u_pot_new"] for r in res], axis=0)
    u_dep_new = np.concatenate([r["u_dep_new"] for r in res], axis=0)
    return (W.copy(), W_new, xbar_new, u_pot_new, u_dep_new), out


def kernel(**inputs):
    outs, _ = _run(inputs, trace=False)
    return outs
